# revision 50
# baseline (speedup 1.0000x reference)
"""Trainium2 Bass kernel: dark-channel + 15x15 erosion (min-pool, stride 1,
+inf padding), data-parallel over 8 NeuronCores.

Input  I: [32, 3, 512, 512] f32, k: scalar (15)
Output:   [32, 1, 512, 512] f32  (min over channels, then kxk spatial min)

Per-core plan (4 images each, pipelined via Tile pools):
  1. DMA the image (3 channels, one transfer) into SBUF, rows on partitions.
  2. Channel min on DVE (2 f32 tensor_tensor min ops, second in place).
  3. Horizontal 15-min-filter on DVE: dyadic shifted mins (1,2,4,7); the
     first stage also converts f32 -> f16.
  4. PE transpose (identity matmul), 4 blocks per PSUM bank, one ScalarE
     evac per bank -> column layout.
  5. Vertical 15-min-filter on DVE (same dyadic trick along free dim).
  6. PE transpose back + ScalarE evac (f16 -> f32 cast) -> row layout.
  7. DMA result to HBM.

fp16 intermediates: values are mins of uniform[0,1) data; min is selection,
not arithmetic, so fp16 keeps rel err ~1e-4.  Pad value 30000.0 acts as
+inf for this data range.

The walrus backend encodes at most ONE sync-wait per instruction and fails
codegen with "Too many sync wait commands" otherwise, while Tile freely
emits several (pool slot reuse, kernel-tail drain).  The post-pass at the
end of _build_nc hoists all but one wait of every instruction onto
single-wait NOPs inserted right before it on the same engine - identical
semantics (the engine sequencer performs the waits in order), and every
instruction then fits the encoding.  CoreSim cannot execute the inserted
NOPs, so the simulator path builds with split_waits=False.
"""

import sys

if "/opt/trn_rl_repo" not in sys.path:
    sys.path.insert(0, "/opt/trn_rl_repo")

import numpy as np

N_CORES = 8
IMGS = 4          # images per core
C = 3
H = W = 512
K = 15
PAD = K // 2      # 7
L = 8             # left pad in filter buffers (>= PAD+1, power of 2)
PITCH = L + 512 + 8   # 528, padded row/col length
NJ = H // 128     # row tiles
NB = W // 128     # col blocks
PADV = 30000.0    # effective +inf for data in [0,1)

_cache = {}


def _build_nc(use_f16=True, split_waits=True):
    import concourse.bass as bass
    import concourse.mybir as mybir
    import concourse.tile as tile
    import concourse.masks as masks

    F32 = mybir.dt.float32
    FI = mybir.dt.float16 if use_f16 else F32
    MIN = mybir.AluOpType.min

    nc = bass.Bass("TRN2", target_bir_lowering=False, debug=False)
    inp = nc.dram_tensor("inp", [IMGS, C, H, W], F32, kind="ExternalInput")
    out = nc.dram_tensor("out", [IMGS, 1, H, W], F32, kind="ExternalOutput")

    def dyadic(pool, src, n):
        """15-wide min filter along last dim of src [128, n, PITCH];
        logical x at [L : L+512].  Returns [128, n, 512] f16.  The first
        stage converts src's dtype (f32 for the h-pass) to f16."""
        f2 = pool.tile([128, n, PITCH], FI, tag="fa", name="f2")
        nc.vector.tensor_tensor(
            f2[:, :, 0:526], src[:, :, 0:526], src[:, :, 1:527], op=MIN
        )
        f4 = pool.tile([128, n, PITCH], FI, tag="fb", name="f4")
        nc.vector.tensor_tensor(
            f4[:, :, 0:524], f2[:, :, 0:524], f2[:, :, 2:526], op=MIN
        )
        f8 = pool.tile([128, n, PITCH], FI, tag="fa", name="f8")
        nc.vector.tensor_tensor(
            f8[:, :, 0:520], f4[:, :, 0:520], f4[:, :, 4:524], op=MIN
        )
        res = pool.tile([128, n, 512], FI, tag="res", name="res")
        nc.vector.tensor_tensor(
            res[:], f8[:, :, 1:513], f8[:, :, 8:520], op=MIN
        )
        return res

    with tile.TileContext(nc) as tc:
        with (
            tc.tile_pool(name="const", bufs=1) as cpool,
            tc.tile_pool(name="io", bufs=3) as io_pool,
            tc.tile_pool(name="work", bufs=2) as work,
            tc.tile_pool(name="resw", bufs=3) as resw,
            tc.tile_pool(name="opool", bufs=2) as opool,
            tc.tile_pool(name="psum", bufs=8, space="PSUM") as psum,
        ):
            ident = cpool.tile([128, 128], FI)
            masks.make_identity(nc, ident[:])

            for i in range(IMGS):
                # --- load: one DMA; (c j w) merges on both sides
                in_t = io_pool.tile([128, C, NJ, W], F32, name="in_t")
                nc.sync.dma_start(
                    in_t[:], inp[i].rearrange("c (j p) w -> p c j w", p=128)
                )

                # --- channel min (GpSimd), all f32 (Pool ucode does not
                # convert dtypes); result lands in a padded f32 buffer.
                xpad = work.tile([128, NJ, PITCH], F32, tag="xp",
                                 name="xpad")
                nc.gpsimd.memset(xpad[:, :, 0:L], PADV)
                nc.gpsimd.memset(xpad[:, :, L + W : PITCH], PADV)
                nc.vector.tensor_tensor(
                    xpad[:, :, L : L + W], in_t[:, 0, :, :],
                    in_t[:, 1, :, :], op=MIN
                )
                nc.vector.tensor_tensor(
                    xpad[:, :, L : L + W], xpad[:, :, L : L + W],
                    in_t[:, 2, :, :], op=MIN
                )

                # --- horizontal filter (DVE; f32 in, f16 out)
                r = dyadic(resw, xpad, NJ)

                # --- transpose to column layout; 4 blocks (all j for one
                # b) fill one PSUM bank, ONE ACT evac per bank.
                vb = work.tile([128, NB, PITCH], FI, tag="vb", name="vb")
                nc.gpsimd.memset(vb[:, :, 0:L], PADV)
                nc.gpsimd.memset(vb[:, :, L + H : PITCH], PADV)
                for b in range(NB):
                    pt = psum.tile([128, 2 * NJ, 128], FI, tag="pt",
                                   name="pt")
                    for j in range(NJ):
                        nc.tensor.transpose(
                            pt[:, j, :], r[:, j, 128 * b : 128 * (b + 1)],
                            ident[:],
                        )
                    nc.scalar.copy(
                        vb[:, b, L : L + H],
                        pt[:, 0:NJ, :].rearrange("p n w -> p (n w)"),
                    )

                # --- vertical filter (DVE)
                u = dyadic(resw, vb, NB)

                # --- transpose back, f32 out
                o = opool.tile([128, NJ, W], F32, name="o")
                for j in range(NJ):
                    pt = psum.tile([128, 2 * NB, 128], FI, tag="pt",
                                   name="pt")
                    for b in range(NB):
                        nc.tensor.transpose(
                            pt[:, b, :], u[:, b, 128 * j : 128 * (j + 1)],
                            ident[:],
                        )
                    nc.scalar.copy(
                        o[:, j, :],
                        pt[:, 0:NB, :].rearrange("p n w -> p (n w)"),
                    )

                # --- store
                nc.sync.dma_start(
                    out[i, 0].rearrange("(j p) w -> p j w", p=128), o[:]
                )

    if not split_waits:
        return nc
    # Post-pass: walrus encodes at most ONE sync-wait per instruction.
    # Hoist all but one wait of any multi-wait instruction onto
    # single-wait NOPs inserted just before it on the same engine
    # (identical semantics: the sequencer performs the waits in order).
    nsplit = 0
    for bb in nc.main_func.blocks:
        idx = 0
        while idx < len(bb.instructions):
            ins = bb.instructions[idx]
            si = ins.sync_info
            if si is not None and si.on_wait and len(si.on_wait) > 1:
                waits = list(si.on_wait)
                for w in waits[:-1]:
                    nop = mybir.InstNoOp(
                        name=f"W-split-{nsplit}", ins=[], outs=[]
                    )
                    nop.engine = ins.engine
                    nop.sync_info = mybir.SyncInfo(
                        on_wait=[w], on_update=[]
                    )
                    bb.instructions.insert(idx, nop)
                    nsplit += 1
                    idx += 1
                ins.sync_info = mybir.SyncInfo(
                    on_wait=[waits[-1]], on_update=list(si.on_update or [])
                )
            idx += 1
    return nc


def _get_nc():
    if "nc" not in _cache:
        _cache["nc"] = _build_nc()
    return _cache["nc"]


def kernel(I, k):
    from concourse.bass_utils import run_bass_kernel_spmd

    k = int(np.asarray(k))
    assert k == K, f"kernel compiled for k={K}, got {k}"
    I = np.ascontiguousarray(np.asarray(I), dtype=np.float32)
    B = I.shape[0]
    assert I.shape == (B, C, H, W) and B == N_CORES * IMGS

    nc = _get_nc()
    in_maps = [
        {"inp": I[c * IMGS : (c + 1) * IMGS]} for c in range(N_CORES)
    ]
    res = run_bass_kernel_spmd(nc, in_maps, list(range(N_CORES))).results
    return np.concatenate([res[c]["out"] for c in range(N_CORES)], axis=0)


# revision 51
# speedup vs baseline: 1.1511x; 1.1511x over previous
"""Trainium2 Bass kernel: dark-channel + 15x15 erosion (min-pool, stride 1,
+inf padding), data-parallel over 8 NeuronCores.

Input  I: [32, 3, 512, 512] f32, k: scalar (15)
Output:   [32, 1, 512, 512] f32  (min over channels, then kxk spatial min)

Per-core plan (4 images each, pipelined via Tile pools):
  1. DMA the image (3 channels, one transfer) into SBUF, rows on partitions.
  2. Channel min on DVE (2 f32 tensor_tensor min ops, second in place).
  3. Horizontal 15-min-filter on DVE: dyadic shifted mins (1,2,4,7); the
     first stage also converts f32 -> f16.
  4. PE transpose (identity matmul), 4 blocks per PSUM bank, one ScalarE
     evac per bank -> column layout.
  5. Vertical 15-min-filter on DVE (same dyadic trick along free dim).
  6. PE transpose back + ScalarE evac (f16 -> f32 cast) -> row layout.
  7. DMA result to HBM.

fp16 intermediates: values are mins of uniform[0,1) data; min is selection,
not arithmetic, so fp16 keeps rel err ~1e-4.  Pad value 30000.0 acts as
+inf for this data range.

The walrus backend encodes at most ONE sync-wait per instruction and fails
codegen with "Too many sync wait commands" otherwise, while Tile freely
emits several (pool slot reuse, kernel-tail drain).  The post-pass at the
end of _build_nc hoists all but one wait of every instruction onto
single-wait NOPs inserted right before it on the same engine - identical
semantics (the engine sequencer performs the waits in order), and every
instruction then fits the encoding.  CoreSim cannot execute the inserted
NOPs, so the simulator path builds with split_waits=False.
"""

import sys

if "/opt/trn_rl_repo" not in sys.path:
    sys.path.insert(0, "/opt/trn_rl_repo")

import numpy as np

N_CORES = 8
IMGS = 4          # images per core
C = 3
H = W = 512
K = 15
PAD = K // 2      # 7
L = 8             # left pad in filter buffers (>= PAD+1, power of 2)
PITCH = L + 512 + 8   # 528, padded row/col length
NJ = H // 128     # row tiles
NB = W // 128     # col blocks
PADV = 30000.0    # effective +inf for data in [0,1)

_cache = {}


def _build_nc(use_f16=True, split_waits=True):
    import concourse.bass as bass
    import concourse.mybir as mybir
    import concourse.tile as tile
    import concourse.masks as masks

    F32 = mybir.dt.float32
    FI = mybir.dt.float16 if use_f16 else F32
    MIN = mybir.AluOpType.min

    nc = bass.Bass("TRN2", target_bir_lowering=False, debug=False)
    inp = nc.dram_tensor("inp", [IMGS, C, H, W], F32, kind="ExternalInput")
    out = nc.dram_tensor("out", [IMGS, 1, H, W], F32, kind="ExternalOutput")

    def dyadic(pool, src, n):
        """15-wide min filter along last dim of src [128, n, PITCH];
        logical x at [L : L+512].  Returns [128, n, 512] f16.  The first
        stage converts src's dtype (f32 for the h-pass) to f16."""
        f2 = pool.tile([128, n, PITCH], FI, tag="fa", name="f2")
        nc.vector.tensor_tensor(
            f2[:, :, 0:526], src[:, :, 0:526], src[:, :, 1:527], op=MIN
        )
        f4 = pool.tile([128, n, PITCH], FI, tag="fb", name="f4")
        nc.vector.tensor_tensor(
            f4[:, :, 0:524], f2[:, :, 0:524], f2[:, :, 2:526], op=MIN
        )
        f8 = pool.tile([128, n, PITCH], FI, tag="fa", name="f8")
        nc.vector.tensor_tensor(
            f8[:, :, 0:520], f4[:, :, 0:520], f4[:, :, 4:524], op=MIN
        )
        res = pool.tile([128, n, 512], FI, tag="res", name="res")
        nc.vector.tensor_tensor(
            res[:], f8[:, :, 1:513], f8[:, :, 8:520], op=MIN
        )
        return res

    with tile.TileContext(nc) as tc:
        with (
            tc.tile_pool(name="const", bufs=1) as cpool,
            tc.tile_pool(name="io", bufs=3) as io_pool,
            tc.tile_pool(name="work", bufs=2) as work,
            tc.tile_pool(name="resw", bufs=3) as resw,
            tc.tile_pool(name="opool", bufs=2) as opool,
            tc.tile_pool(name="psum", bufs=8, space="PSUM") as psum,
        ):
            ident = cpool.tile([128, 128], FI)
            masks.make_identity(nc, ident[:])

            for i in range(IMGS):
                # --- load: one DMA; (c j w) merges on both sides
                in_t = io_pool.tile([128, C, NJ, W], F32, name="in_t")
                nc.sync.dma_start(
                    in_t[:], inp[i].rearrange("c (j p) w -> p c j w", p=128)
                )

                # --- channel min (GpSimd): min(c0,c1) -> contiguous f32
                # scratch, then min with c2 -> f16 padded buffer (this
                # exact op/dtype/AP pattern is known to pass the walrus
                # Pool engine check).
                scr = work.tile([128, NJ, W], F32, tag="scr", name="scr")
                nc.gpsimd.tensor_tensor(
                    scr[:], in_t[:, 0, :, :], in_t[:, 1, :, :], op=MIN
                )
                xpad = work.tile([128, NJ, PITCH], FI, tag="xp",
                                 name="xpad")
                nc.gpsimd.memset(xpad[:, :, 0:L], PADV)
                nc.gpsimd.memset(xpad[:, :, L + W : PITCH], PADV)
                nc.gpsimd.tensor_tensor(
                    xpad[:, :, L : L + W], scr[:], in_t[:, 2, :, :], op=MIN
                )

                # --- horizontal filter (DVE)
                r = dyadic(resw, xpad, NJ)

                # --- transpose to column layout; 4 blocks (all j for one
                # b) fill one PSUM bank, ONE ACT evac per bank.
                vb = work.tile([128, NB, PITCH], FI, tag="vb", name="vb")
                nc.gpsimd.memset(vb[:, :, 0:L], PADV)
                nc.gpsimd.memset(vb[:, :, L + H : PITCH], PADV)
                for b in range(NB):
                    pt = psum.tile([128, 2 * NJ, 128], FI, tag="pt",
                                   name="pt")
                    for j in range(NJ):
                        nc.tensor.transpose(
                            pt[:, j, :], r[:, j, 128 * b : 128 * (b + 1)],
                            ident[:],
                        )
                    nc.scalar.copy(
                        vb[:, b, L : L + H],
                        pt[:, 0:NJ, :].rearrange("p n w -> p (n w)"),
                    )

                # --- vertical filter (DVE)
                u = dyadic(resw, vb, NB)

                # --- transpose back, f32 out
                o = opool.tile([128, NJ, W], F32, name="o")
                for j in range(NJ):
                    pt = psum.tile([128, 2 * NB, 128], FI, tag="pt",
                                   name="pt")
                    for b in range(NB):
                        nc.tensor.transpose(
                            pt[:, b, :], u[:, b, 128 * j : 128 * (j + 1)],
                            ident[:],
                        )
                    nc.scalar.copy(
                        o[:, j, :],
                        pt[:, 0:NB, :].rearrange("p n w -> p (n w)"),
                    )

                # --- store
                nc.sync.dma_start(
                    out[i, 0].rearrange("(j p) w -> p j w", p=128), o[:]
                )

    if not split_waits:
        return nc
    # Post-pass: walrus encodes at most ONE sync-wait per instruction.
    # Hoist all but one wait of any multi-wait instruction onto
    # single-wait NOPs inserted just before it on the same engine
    # (identical semantics: the sequencer performs the waits in order).
    nsplit = 0
    for bb in nc.main_func.blocks:
        idx = 0
        while idx < len(bb.instructions):
            ins = bb.instructions[idx]
            si = ins.sync_info
            if si is not None and si.on_wait and len(si.on_wait) > 1:
                waits = list(si.on_wait)
                for w in waits[:-1]:
                    nop = mybir.InstNoOp(
                        name=f"W-split-{nsplit}", ins=[], outs=[]
                    )
                    nop.engine = ins.engine
                    nop.sync_info = mybir.SyncInfo(
                        on_wait=[w], on_update=[]
                    )
                    bb.instructions.insert(idx, nop)
                    nsplit += 1
                    idx += 1
                ins.sync_info = mybir.SyncInfo(
                    on_wait=[waits[-1]], on_update=list(si.on_update or [])
                )
            idx += 1
    return nc


def _get_nc():
    if "nc" not in _cache:
        _cache["nc"] = _build_nc()
    return _cache["nc"]


def kernel(I, k):
    from concourse.bass_utils import run_bass_kernel_spmd

    k = int(np.asarray(k))
    assert k == K, f"kernel compiled for k={K}, got {k}"
    I = np.ascontiguousarray(np.asarray(I), dtype=np.float32)
    B = I.shape[0]
    assert I.shape == (B, C, H, W) and B == N_CORES * IMGS

    nc = _get_nc()
    in_maps = [
        {"inp": I[c * IMGS : (c + 1) * IMGS]} for c in range(N_CORES)
    ]
    res = run_bass_kernel_spmd(nc, in_maps, list(range(N_CORES))).results
    return np.concatenate([res[c]["out"] for c in range(N_CORES)], axis=0)
